# revision 1
# baseline (speedup 1.0000x reference)
"""Trainium2 Bass kernel for nn_CausalPerformer (causal linear attention).

Self-contained: kernel(**inputs) -> np.ndarray.

Strategy (8 NeuronCores, SPMD over sequence rows):
  - Flatten (B,S) -> 8192 rows; core i owns rows [i*1024, (i+1)*1024).
  - Host prep: transpose+cast activations to bf16; fuse omega into W_q/W_k
    (x = q @ (omega@W_q).T), so only V and O need full 1024x1024 projections.
  - q' normalization cancels in num/den -> skipped. k' normalization folds
    into a scaled V matrix (Vtil = [vh*recip | recip]).
  - Launch 1 (kernel1): k features, V projection, per-chunk state deltas.
  - Host: tiny exclusive prefix-sum of the (7x65) chunk states per (b,h).
  - Launch 2 (kernel2): q features, masked intra-chunk attention + state
    term, divide, output projection.
"""

import numpy as np
import ml_dtypes

import concourse.bacc as bacc
import concourse.mybir as mybir
from concourse import tile
from concourse.bass_utils import run_bass_kernel_spmd

BF16 = mybir.dt.bfloat16
F32 = mybir.dt.float32
NPBF16 = ml_dtypes.bfloat16

B, S, D = 2, 4096, 1024
H, DK, F = 16, 64, 7
NC = 8
RPC = B * S // NC          # 1024 rows per core
CH = 128                   # chunk length
NCH = RPC // CH            # 8 chunks per core
NDT = D // 128             # 8 D-chunks
EPS = 1e-6
ACT_EXP = mybir.ActivationFunctionType.Exp
ACT_SQ = mybir.ActivationFunctionType.Square
ACT_COPY = mybir.ActivationFunctionType.Copy

_cache = {}


def _bacc():
    return bacc.Bacc("TRN2", target_bir_lowering=False, debug=False, num_devices=NC)


def build_kernel1(repeat=1):
    nc = _bacc()
    kT = nc.dram_tensor("kT", [D, RPC], BF16, kind="ExternalInput")
    vT = nc.dram_tensor("vT", [D, RPC], BF16, kind="ExternalInput")
    wfk_pad = nc.dram_tensor("wfk_pad", [4, 128, NDT * 128], BF16, kind="ExternalInput")
    wfk_nat = nc.dram_tensor("wfk_nat", [128, NDT * 112], BF16, kind="ExternalInput")
    wvT = nc.dram_tensor("wvT", [D, D], BF16, kind="ExternalInput")
    kpt = nc.dram_tensor("kpt", [128, 4 * RPC], BF16, kind="ExternalOutput")
    vtil = nc.dram_tensor("vtil", [128, NCH * H * 65], BF16, kind="ExternalOutput")
    st = nc.dram_tensor("st", [128, NCH * 4 * 65], F32, kind="ExternalOutput")

    with tile.TileContext(nc) as tc:
        with (
            tc.tile_pool(name="pers", bufs=1) as pers,
            tc.tile_pool(name="work", bufs=2) as work,
            tc.tile_pool(name="ps", bufs=2, space="PSUM") as ps,
        ):
            wfkp_all = pers.tile([128, 4 * NDT * 128], BF16, tag="wfkp")
            nc.sync.dma_start(
                out=wfkp_all[:].rearrange("p (j n) -> p j n", j=4),
                in_=wfk_pad.ap().rearrange("j p n -> p j n"))
            wfkn_sb = pers.tile([128, NDT * 112], BF16, tag="wfkn")
            nc.sync.dma_start(out=wfkn_sb[:], in_=wfk_nat.ap()[:, :])
            kt_all = pers.tile([128, NDT * RPC], BF16, tag="kt")
            for sh in range(2):
                nc.sync.dma_start(
                    out=kt_all[:].rearrange("p (c s) -> p c s", c=NDT)[
                        :, :, sh * 512:(sh + 1) * 512],
                    in_=kT.ap().rearrange("(c p) s -> p c s", p=128)[
                        :, :, sh * 512:(sh + 1) * 512])
            vt_all = pers.tile([128, NDT * RPC], BF16, tag="vt")
            nc.sync.dma_start(
                out=vt_all[:].rearrange("p (c s) -> p c s", c=NDT),
                in_=vT.ap().rearrange("(c p) s -> p c s", p=128))
            wv_all = pers.tile([128, NDT * D], BF16, tag="wv")
            nc.sync.dma_start(
                out=wv_all[:].rearrange("p (c n) -> p c n", c=NDT),
                in_=wvT.ap().rearrange("(c p) n -> p c n", p=128))

            def kts(c):
                return kt_all[:, c * RPC:(c + 1) * RPC]

            def vtc(c):
                return vt_all[:, c * RPC:(c + 1) * RPC]

            for _rep in range(repeat):
                # ---- phase A: transposed k' feature tiles (4h x 32 rows) ----
                kpt_sb = work.tile([128, 4 * RPC], BF16, tag="kptsb", bufs=1)
                for j in range(4):
                    for hf in range(2):
                        pf = ps.tile([128, 512], F32, tag="kft", bufs=2)
                        for c in range(NDT):
                            nc.tensor.matmul(
                                pf[:],
                                wfkp_all[:, (j * NDT + c) * 128:(j * NDT + c + 1) * 128],
                                kts(c)[:, hf * 512:(hf + 1) * 512],
                                start=(c == 0), stop=(c == NDT - 1),
                            )
                        sq = work.tile([128, 512], F32, tag="sq")
                        nc.scalar.activation(sq[:], pf[:], ACT_SQ)
                        nc.scalar.activation(
                            kpt_sb[:, j * RPC + hf * 512:j * RPC + (hf + 1) * 512],
                            sq[:], ACT_EXP, scale=-0.5)
                nc.sync.dma_start(out=kpt.ap()[:, :], in_=kpt_sb[:])

                # ---- phase B: per chunk: k'nat, recip, vh, Vtil, deltas ----
                for t in range(NCH):
                    tsl = slice(t * 128, (t + 1) * 128)
                    pkn = ps.tile([128, 112], F32, tag="kn", bufs=2)
                    for c in range(NDT):
                        nc.tensor.matmul(
                            pkn[:],
                            kts(c)[:, tsl],
                            wfkn_sb[:, c * 112:(c + 1) * 112],
                            start=(c == 0), stop=(c == NDT - 1),
                        )
                    sqn = work.tile([128, 112], F32, tag="sqn")
                    nc.scalar.activation(sqn[:], pkn[:], ACT_SQ)
                    kexp = work.tile([128, 112], F32, tag="kexp")
                    nc.scalar.activation(kexp[:], sqn[:], ACT_EXP, scale=-0.5)
                    knb = work.tile([128, 112], BF16, tag="knb")
                    nc.scalar.activation(knb[:], kexp[:], ACT_COPY)
                    rc = work.tile([128, 16], F32, tag="rc")
                    nc.vector.reduce_sum(
                        rc[:], kexp[:].rearrange("p (h f) -> p h f", f=F),
                        axis=mybir.AxisListType.X,
                    )
                    nc.vector.tensor_scalar_add(rc[:], rc[:], EPS)
                    nc.vector.reciprocal(rc[:], rc[:])

                    vts = work.tile([128, H * 65], BF16, tag="vts")
                    for hf in range(2):
                        pv = ps.tile([128, 512], F32, tag="vh", bufs=3)
                        for c in range(NDT):
                            nc.tensor.matmul(
                                pv[:],
                                vtc(c)[:, tsl],
                                wv_all[:, c * D + hf * 512:c * D + (hf + 1) * 512],
                                start=(c == 0), stop=(c == NDT - 1),
                            )
                        for hh in range(8):
                            h = hf * 8 + hh
                            nc.vector.tensor_scalar_mul(
                                vts[:, h * 65:h * 65 + 64],
                                pv[:, hh * 64:(hh + 1) * 64],
                                rc[:, h:h + 1],
                            )
                    nc.vector.tensor_copy(
                        vts[:].rearrange("p (h n) -> p h n", n=65)[:, :, 64:65],
                        rc[:].rearrange("p (h o) -> p h o", o=1),
                    )
                    nc.sync.dma_start(
                        out=vtil.ap()[:, t * H * 65:(t + 1) * H * 65], in_=vts[:])

                    sts = work.tile([128, 4 * 65], F32, tag="sts")
                    for j in range(4):
                        pst = ps.tile([128, 65], F32, tag="st", bufs=1)
                        for g in range(4):
                            h = 4 * j + g
                            nc.tensor.matmul(
                                pst[32 * g:32 * g + 7, :],
                                knb[:, 7 * h:7 * h + 7],
                                vts[:, 65 * h:65 * h + 65],
                                start=True, stop=True,
                                tile_position=(0, 32 * g),
                            )
                        nc.vector.tensor_copy(sts[:, j * 65:(j + 1) * 65], pst[:])
                    nc.sync.dma_start(
                        out=st.ap()[:, t * 260:(t + 1) * 260], in_=sts[:])

    nc.compile()
    return nc


def build_kernel2(repeat=1):
    nc = _bacc()
    qT = nc.dram_tensor("qT", [D, RPC], BF16, kind="ExternalInput")
    wfq_pad = nc.dram_tensor("wfq_pad", [4, 128, NDT * 128], BF16, kind="ExternalInput")
    kpt = nc.dram_tensor("kpt", [128, 4 * RPC], BF16, kind="ExternalInput")
    vtil = nc.dram_tensor("vtil", [128, NCH * H * 65], BF16, kind="ExternalInput")
    stbd = nc.dram_tensor("stbd", [128, NCH * 4 * 260], BF16, kind="ExternalInput")
    woT = nc.dram_tensor("woT", [D, D], BF16, kind="ExternalInput")
    consts = nc.dram_tensor("consts", [128, 256], BF16, kind="ExternalInput")
    o = nc.dram_tensor("o", [RPC, D], F32, kind="ExternalOutput")

    with tile.TileContext(nc) as tc:
        with (
            tc.tile_pool(name="pers", bufs=1) as pers,
            tc.tile_pool(name="work", bufs=2) as work,
        ):
            cst = pers.tile([128, 256], BF16, tag="cst")
            nc.sync.dma_start(out=cst[:], in_=consts.ap()[:, :])
            wfqp_all = pers.tile([128, 4 * NDT * 128], BF16, tag="wfqp")
            nc.sync.dma_start(
                out=wfqp_all[:].rearrange("p (j n) -> p j n", j=4),
                in_=wfq_pad.ap().rearrange("j p n -> p j n"))
            qt_all = pers.tile([128, NDT * RPC], BF16, tag="qt")
            for sh in range(2):
                nc.sync.dma_start(
                    out=qt_all[:].rearrange("p (c s) -> p c s", c=NDT)[
                        :, :, sh * 512:(sh + 1) * 512],
                    in_=qT.ap().rearrange("(c p) s -> p c s", p=128)[
                        :, :, sh * 512:(sh + 1) * 512])
            kpt_all = pers.tile([128, 4 * RPC], BF16, tag="kpt")
            nc.sync.dma_start(out=kpt_all[:], in_=kpt.ap()[:, :])
            vt_all = pers.tile([128, NCH * H * 65], BF16, tag="vtl")
            nc.sync.dma_start(out=vt_all[:], in_=vtil.ap()[:, :])
            st_all = pers.tile([128, NCH * 4 * 260], BF16, tag="st")
            nc.sync.dma_start(out=st_all[:], in_=stbd.ap()[:, :])
            wo_all = pers.tile([128, NDT * D], BF16, tag="wo")
            nc.sync.dma_start(
                out=wo_all[:].rearrange("p (c n) -> p c n", c=NDT),
                in_=woT.ap().rearrange("(c p) n -> p c n", p=128))
            mask_tri = cst[:, 0:128]
            ident = cst[:, 128:256]

            for _rep in range(repeat):
                # ---- q' feature tiles ----
                qpt_sb = work.tile([128, 4 * RPC], BF16, tag="qpt", bufs=1)
                with tc.tile_pool(name="psq", bufs=2, space="PSUM") as psq:
                    for j in range(4):
                        for hf in range(2):
                            pf = psq.tile([128, 512], F32, tag="qft")
                            for c in range(NDT):
                                nc.tensor.matmul(
                                    pf[:],
                                    wfqp_all[:, (j * NDT + c) * 128:(j * NDT + c + 1) * 128],
                                    qt_all[:, c * RPC + hf * 512:c * RPC + (hf + 1) * 512],
                                    start=(c == 0), stop=(c == NDT - 1),
                                )
                            sq = work.tile([128, 512], F32, tag="sq")
                            nc.scalar.activation(sq[:], pf[:], ACT_SQ)
                            nc.scalar.activation(
                                qpt_sb[:, j * RPC + hf * 512:j * RPC + (hf + 1) * 512],
                                sq[:], ACT_EXP, scale=-0.5)

                def qpt(j):
                    return qpt_sb[:, j * RPC:(j + 1) * RPC]

                def kptj(j):
                    return kpt_all[:, j * RPC:(j + 1) * RPC]

                # ---- attention chunks ----
                with tc.tile_pool(name="psm", bufs=1, space="PSUM") as psm:
                    for t in range(NCH):
                        tsl = slice(t * 128, (t + 1) * 128)
                        # AT: 4 heads of tile j, one PSUM bank each
                        # (bank-aligned 512-elem offsets), masked in one op
                        atm = []
                        for j in range(4):
                            pat4 = psm.tile([128, 4 * 512], F32, tag="at",
                                            bufs=1, name="pat4")
                            for g in range(4):
                                nc.tensor.matmul(
                                    pat4[:, 512 * g:512 * g + 128],
                                    kptj(j)[32 * g:32 * g + 7, tsl],
                                    qpt(j)[32 * g:32 * g + 7, tsl],
                                    start=True, stop=True,
                                    tile_position=(32 * g, 0),
                                )
                            am4 = work.tile([128, 512], BF16, tag="am", bufs=3,
                                            name="am4")
                            nc.vector.tensor_mul(
                                am4[:].rearrange("p (g q) -> p g q", q=128),
                                pat4[:].rearrange("p (g q) -> p g q", q=512)[
                                    :, :, 0:128],
                                mask_tri[:, None, :].broadcast_to([128, 4, 128]),
                            )
                            atm.append(am4)
                        # numerators per j-tile: block-diag inter mm first,
                        # then per-head intra mms close each 65-col group
                        den = work.tile([128, 16], F32, tag="den", bufs=2)
                        oh_all = work.tile([128, H * DK], BF16, tag="oh", bufs=2)
                        for j in range(4):
                            pnj = psm.tile([128, 260], F32, tag="num", bufs=2,
                                           name="pnj")
                            nc.tensor.matmul(
                                pnj[:],
                                qpt(j)[:, tsl],
                                st_all[:, (t * 4 + j) * 260:(t * 4 + j + 1) * 260],
                                start=True, stop=False,
                                skip_group_check=True,
                            )
                            for g in range(4):
                                h = 4 * j + g
                                nc.tensor.matmul(
                                    pnj[:, 65 * g:65 * (g + 1)],
                                    atm[j][:, 128 * g:128 * (g + 1)],
                                    vt_all[:, (t * H + h) * 65:(t * H + h + 1) * 65],
                                    start=False, stop=True,
                                    skip_group_check=True,
                                )
                            dj = den[:, 4 * j:4 * j + 4]
                            nc.vector.tensor_scalar_add(
                                dj.rearrange("p (h o) -> p h o", o=1),
                                pnj[:].rearrange(
                                    "p (h n) -> p h n", n=65)[:, :, 64:65],
                                EPS,
                            )
                            nc.vector.reciprocal(dj, dj)
                            nc.vector.tensor_mul(
                                oh_all[:, 256 * j:256 * (j + 1)].rearrange(
                                    "p (h d) -> p h d", d=64),
                                pnj[:].rearrange(
                                    "p (h n) -> p h n", n=65)[:, :, 0:64],
                                dj[:, :, None].broadcast_to([128, 4, 64]),
                            )
                        # transpose head pairs -> lhsT tiles for O-projection
                        ohT = []
                        for p in range(8):
                            ptr = psm.tile([128, 128], BF16, tag="tr", bufs=1,
                                           name=f"ptr{p}")
                            nc.tensor.transpose(
                                ptr[:], oh_all[:, 128 * p:128 * (p + 1)],
                                ident)
                            otr = work.tile([128, 128], BF16, tag="otr", bufs=8,
                                            name=f"otr{p}")
                            nc.vector.tensor_copy(otr[:], ptr[:])
                            ohT.append(otr)
                        # output projection
                        osb = work.tile([128, D], F32, tag="osb", bufs=2)
                        for hf in range(2):
                            po = psm.tile([128, 512], F32, tag="po", bufs=1)
                            for p in range(8):
                                nc.tensor.matmul(
                                    po[:],
                                    ohT[p][:],
                                    wo_all[:, p * D + hf * 512:p * D + (hf + 1) * 512],
                                    start=(p == 0), stop=(p == 7),
                                )
                            nc.scalar.activation(
                                osb[:, hf * 512:(hf + 1) * 512], po[:], ACT_COPY)
                        nc.sync.dma_start(out=o.ap()[tsl, :], in_=osb[:])

    nc.compile()
    return nc


def _host_prep(q, k, v, w_q, w_k, w_v, w_o, omega):
    """Host-side input marshaling: transposes, casts, weight fusion."""
    Wfq = np.einsum("fd,hdD->hfD", omega, w_q.reshape(H, DK, D)).reshape(H * F, D)
    Wfk = np.einsum("fd,hdD->hfD", omega, w_k.reshape(H, DK, D)).reshape(H * F, D)

    def pad_tiles(Wf):
        # [4, 128p(D-in-chunk), 8c*128m] lhsT tiles; col 32g+f = Wf[(4j+g)*7+f]
        out = np.zeros((4, 128, NDT * 128), np.float32)
        for j in range(4):
            wt = np.zeros((D, 128), np.float32)
            for g in range(4):
                wt[:, 32 * g:32 * g + 7] = Wf[(4 * j + g) * 7:(4 * j + g) * 7 + 7].T
            out[j] = wt.reshape(NDT, 128, 128).transpose(1, 0, 2).reshape(128, NDT * 128)
        return out.astype(NPBF16)

    wfq_pad = pad_tiles(Wfq)
    wfk_pad = pad_tiles(Wfk)
    # k'nat rhs: [128p, 8c*112]; chunk c cols = Wfk.T[c*128:(c+1)*128, :]
    wfk_nat = (
        Wfk.T.reshape(NDT, 128, H * F).transpose(1, 0, 2).reshape(128, NDT * H * F)
    ).astype(NPBF16)
    wvT = np.ascontiguousarray(w_v.T).astype(NPBF16)
    woT = np.ascontiguousarray(w_o.T).astype(NPBF16)

    qf = q.reshape(B * S, D)
    kf = k.reshape(B * S, D)
    vf = v.reshape(B * S, D)
    qT, kT, vT = [], [], []
    for i in range(NC):
        rows = slice(i * RPC, (i + 1) * RPC)
        qT.append(np.ascontiguousarray(qf[rows].T).astype(NPBF16))
        kT.append(np.ascontiguousarray(kf[rows].T).astype(NPBF16))
        vT.append(np.ascontiguousarray(vf[rows].T).astype(NPBF16))

    consts = np.zeros((128, 256), np.float32)
    consts[:, 0:128] = np.arange(CH)[:, None] <= np.arange(CH)[None, :]
    consts[:, 128:256] = np.eye(128)
    consts = consts.astype(NPBF16)  # causal mask | identity
    return dict(
        wfq_pad=wfq_pad, wfk_pad=wfk_pad, wfk_nat=wfk_nat, wvT=wvT, woT=woT,
        qT=qT, kT=kT, vT=vT, consts=consts,
    )


def _host_prefix(st_cores):
    """Exclusive prefix over chunk deltas -> block-diag per-chunk input states.

    st_cores[i]: [128, NCH*4*65] f32; view [128, t, j, n]; rows 32g+f of
    block j = head 4j+g.  Returns stbd[i]: [128, NCH*4*260] bf16 where
    block (t, j) is [128, 260] block-diagonal: rows 32g+f, cols 65g+n.
    """
    st = np.stack([np.asarray(s, np.float32) for s in st_cores])
    st = st.reshape(NC, 128, NCH, 4, 65).transpose(0, 2, 1, 3, 4)
    # -> [i, t, p, j, n]; global chunk axis per b:
    st = st.reshape(2, 4 * NCH, 128, 4, 65)
    pref = (np.cumsum(st, axis=1, dtype=np.float64) - st).astype(np.float32)
    pref = pref.reshape(NC, NCH, 128, 4, 65)
    stbd = np.zeros((NC, NCH, 4, 128, 4, 65), np.float32)  # [i,t,j,p,g,n]
    for g in range(4):
        stbd[:, :, :, 32 * g:32 * g + 7, g, :] = \
            pref[:, :, 32 * g:32 * g + 7, :, :].transpose(0, 1, 3, 2, 4)
    # [i,t,j,p,(g n)] -> [i, p, (t j g n)]
    out = stbd.reshape(NC, NCH, 4, 128, 260).transpose(0, 3, 1, 2, 4).reshape(
        NC, 128, NCH * 4 * 260)
    return [np.ascontiguousarray(out[i]).astype(NPBF16) for i in range(NC)]


def kernel(q, k, v, w_q, w_k, w_v, w_o, omega):
    q = np.asarray(q, np.float32)
    k = np.asarray(k, np.float32)
    v = np.asarray(v, np.float32)
    w_q = np.asarray(w_q, np.float32)
    w_k = np.asarray(w_k, np.float32)
    w_v = np.asarray(w_v, np.float32)
    w_o = np.asarray(w_o, np.float32)
    omega = np.asarray(omega, np.float32)

    hp = _host_prep(q, k, v, w_q, w_k, w_v, w_o, omega)

    if "nc1" not in _cache:
        _cache["nc1"] = build_kernel1()
    nc1 = _cache["nc1"]
    in1 = [
        dict(kT=hp["kT"][i], vT=hp["vT"][i], wfk_pad=hp["wfk_pad"],
             wfk_nat=hp["wfk_nat"], wvT=hp["wvT"])
        for i in range(NC)
    ]
    r1 = run_bass_kernel_spmd(nc1, in1, core_ids=list(range(NC)))
    stbd = _host_prefix([r1.results[i]["st"] for i in range(NC)])

    if "nc2" not in _cache:
        _cache["nc2"] = build_kernel2()
    nc2 = _cache["nc2"]
    in2 = [
        dict(qT=hp["qT"][i], wfq_pad=hp["wfq_pad"], kpt=r1.results[i]["kpt"],
             vtil=r1.results[i]["vtil"], stbd=stbd[i], woT=hp["woT"],
             consts=hp["consts"])
        for i in range(NC)
    ]
    r2 = run_bass_kernel_spmd(nc2, in2, core_ids=list(range(NC)))
    out = np.concatenate([r2.results[i]["o"] for i in range(NC)], axis=0)
    return out.reshape(B, S, D)

